# revision 14
# baseline (speedup 1.0000x reference)
"""Trainium2 Bass kernel for nn_CustomBSplineLayer.

Math: out[b,o] = sum_{i,g} coeff[o,i,g] * w[o,i] * s_g(clip(x[b,i], -1, 1))
where s_g is a cubic B-spline basis on uniform knots (t = 3.5*(x+1) in [0,7],
8 basis functions; s_7 == 0 on the clipped domain).

Uniform-knot truncated-power identity: with V_q = relu(t-q)^3, the layer is
out = sum_{q,i} P_q[b,i] * H[(q,i), o] for ANY plane basis P spanning {V_q}
(H solved exactly on host).  The PE runs float32r (full rate) which rounds
each product at ~2^-12 relative, so per-plane error scales with |P_q|*|H_q|.
First-difference planes D1_q = V_q - V_{q+1} (<=127; V_7 := 0) measure ~5e-3
relative output error -- well under the 2e-2 gate -- and have a key property:

    D1_q is a function of a_q = relu(t-q) ALONE:
        D1_q = m^3 + 3*(a_q - m)*a_q,   m = min(a_q, 1)
    (for t>=q+1 this is 3s^2-3s+1 with s=t-q; for t in [q,q+1] it's s^3).

So each plane needs exactly TWO on-chip ops: one ScalarE relu (free bias
shift) and one fused custom DVE instruction (D1CUBE_ANT, registered below,
5 ALU stages).  No folds, no gpsimd, no squares: per i-block the old kernel's
~30 elementwise ops become 14.  The clamp tp = min(3.5x, 3.5) is precomputed
on host (t<0 needs no clamp: every plane vanishes there via the relu).

Layout (data-parallel over batch, 8 cores x 1024 rows):
  - xt = host tp, pre-transposed: [512 i, 1024 b] per core, i on partitions.
  - planes per (i-block, q): [128, 1024] f32r tiles; matmul lhsT slices are
    [128 K, 128 M=batch] column windows; rhs H tiles [128, 512 o] (f32r).
  - h2 DMA'd in 28 per-kt chunks so the first matmul can start ~1us in.
  - PSUM [128 b, 512 o] x 8 banks accumulate all 28 k-tiles.
"""

import numpy as np

import concourse.mybir as mybir
from concourse import bacc
import concourse.tile as tile
from concourse.bass_utils import run_bass_kernel_spmd
from concourse import dve_ops as _dops
from concourse.dve_spec import Spec, Src0, C0, One, minn, sq
from concourse.dve_spec import lower as _dve_lower
from concourse.dve_uop import DveOpSpec as _DveOpSpec

F32 = mybir.dt.float32
F32R = mybir.dt.float32r
ACTF = mybir.ActivationFunctionType

N_CORES = 8
BATCH, I, O, G = 8192, 512, 512, 8
BC = BATCH // N_CORES          # 1024 batch rows per core
Q = 7                          # planes q = 0..6
IB = I // 128                  # 4 i-blocks
KT = Q * IB                    # 28 k-tiles of 128
NBB = BC // 128                # 8 batch blocks of 128


def _register_d1cube():
    """Register the fused plane op: out = m^3 + s0*(a-m)*a, m = min(a, 1).

    With a = relu(t-q) >= 0 and s0 = 3.0 this is exactly
    D1_q(t) = relu(t-q)^3 - relu(t-q-1)^3 for t <= q+... (all t; V_{q+1}
    is a function of a_q since relu(t-q-1) = relu(a_q - 1))."""
    name = "D1CUBE_ANT"
    for op in _dops.OPS:
        if op.name == name:
            return op

    def _ref(in0, in1, s0, s1, imm2):
        a = in0.astype(np.float32)
        m = np.minimum(a, np.float32(1.0))
        return (m * m * m + (a - m) * a * np.float32(s0)).astype(np.float32)

    m = minn(Src0, One)
    spec = Spec(body=sq(m) * m + (Src0 - m) * Src0 * C0, reference=_ref)
    opcode = _dops._CUSTOM_DVE_ROW_BASE + len(_dops.OPS)
    assert opcode < 0x20
    shas = {}
    for ver in ("v3", "v4"):
        try:
            shas[ver] = _DveOpSpec(
                name=name, opcode=opcode, uops=_dve_lower(spec, ver=ver),
                rd1_en=False).sha(ver)
        except Exception:
            pass
    op = _dops.DveOp(name, spec, subdim=False, uops_sha=shas)
    _dops.OPS.append(op)
    _dops.CUSTOM_DVE_SPECS[name] = spec
    _dops._SUB_OPCODE_FOR_NAME[name] = opcode
    return op


D1CUBE = _register_d1cube()

_programs = {}


def _build_program():
    nc = bacc.Bacc("TRN2", target_bir_lowering=False, debug=False,
                   num_devices=N_CORES)
    xt_d = nc.dram_tensor("xt", [I, BC], F32, kind="ExternalInput").ap()
    h2_d = nc.dram_tensor("h2", [KT * 128, O], F32R, kind="ExternalInput").ap()
    qb_d = nc.dram_tensor("qb", [128, 8], F32, kind="ExternalInput").ap()
    out_d = nc.dram_tensor("out", [BC, O], F32, kind="ExternalOutput").ap()

    with tile.TileContext(nc) as tc:
        with tc.tile_pool(name="g", bufs=1) as gpool, \
             tc.tile_pool(name="x", bufs=4) as xpool, \
             tc.tile_pool(name="a", bufs=4) as apool, \
             tc.tile_pool(name="p", bufs=9) as ppool, \
             tc.tile_pool(name="o", bufs=4) as opool, \
             tc.tile_pool(name="ps", bufs=1, space="PSUM") as pspool:

            # warm tile: memset'd (no DMA dep) -- feeds the scalar table-load
            # hoist and the PE HAM warm-up matmuls before real data lands.
            # f32r so the dummies are single-pass (fp32 lowers to 2x LOW_HIGH).
            warm0 = gpool.tile([128, 512], F32)
            nc.gpsimd.memset(warm0[:], 0.0)
            warm = gpool.tile([128, 512], F32R)
            nc.vector.tensor_copy(out=warm[:], in_=warm0[:])
            scr = gpool.tile([128, 8], F32)

            qb_s = gpool.tile([128, 8], F32)
            xs = [xpool.tile([128, BC], F32, name=f"xs{ib}", tag=f"xs{ib}")
                  for ib in range(IB)]

            # scalar queue (q10) carries ONLY xs0, as two column-halves so
            # the A-half pipeline starts ~1.5us before the full tile lands;
            # the dummy activation pulls ACT_TABLE_LOAD ahead while xs0 flies.
            nc.scalar.dma_start(out=xs[0][:, 0:BC // 2],
                                in_=xt_d[0:128, 0:BC // 2])
            nc.scalar.dma_start(out=xs[0][:, BC // 2:BC],
                                in_=xt_d[0:128, BC // 2:BC])
            nc.scalar.activation(scr[:], warm0[:, 0:8], ACTF.Relu, scale=1.0)

            psums = [pspool.tile([128, O], F32, name=f"ps{bb}", tag=f"ps{bb}")
                     for bb in range(NBB)]

            # PE HAM warm-up: ~10 single-pass f32r dummies (~4.3us cold) keep
            # the PE busy from ~7.2us until the first real matmul, so the
            # clock gate opens at ~10.6us and the real stream runs at 2.4GHz.
            for _ in range(8):
                nc.tensor.matmul(psums[0][:], warm[:, 0:128], warm[:],
                                 start=True, stop=True)

            # sync queue: h2 finely chunked up front, then coarse; xs1-3
            # interleaved where their deadlines fall.
            h2_s = gpool.tile([128, KT, O], F32R)

            def h2_dma(k0, k1):
                nc.sync.dma_start(
                    out=h2_s[:, k0:k1, :],
                    in_=h2_d[k0 * 128:k1 * 128, :].rearrange(
                        "(kt p) o -> p kt o", p=128))

            nc.sync.dma_start(out=qb_s[:], in_=qb_d[:])
            h2_dma(0, 1)
            h2_dma(1, 2)
            nc.sync.dma_start(out=xs[1][:], in_=xt_d[128:256, :])
            h2_dma(2, 7)
            nc.sync.dma_start(out=xs[2][:], in_=xt_d[256:384, :])
            nc.sync.dma_start(out=xs[3][:], in_=xt_d[384:512, :])
            h2_dma(7, 14)
            h2_dma(14, 21)
            h2_dma(21, 28)

            # i-blocks 0..2: plane-major (kt inner order), PSUM-bank inner.
            planes = {}
            # i-block 0 as two independent half-pipelines (A: psum banks 0-3,
            # B: banks 4-7).  Bank accumulation only needs kt0 first PER BANK,
            # so the A half streams matmuls while the B half of xs0 is still
            # in flight.
            for hh in range(2):
                cols = slice(hh * BC // 2, (hh + 1) * BC // 2)
                for q in range(Q):
                    ah = apool.tile([128, BC // 2], F32, tag="a0", bufs=3)
                    nc.scalar.activation(ah[:], xs[0][:, cols], ACTF.Relu,
                                         bias=qb_s[:, q:q + 1], scale=1.0)
                    ph = ppool.tile([128, BC // 2], F32R, tag="p0", bufs=3)
                    nc.vector._custom_dve(D1CUBE, out=ph[:], in0=ah[:], s0=3.0)
                    for b4 in range(4):
                        bb = hh * 4 + b4
                        nc.tensor.matmul(psums[bb][:],
                                         ph[:, b4 * 128:(b4 + 1) * 128],
                                         h2_s[:, q, :],
                                         start=(q == 0), stop=False)

            # i-blocks 1..2: plane-major (kt inner order), PSUM-bank inner.
            for ib in range(1, IB):
                for q in range(Q):
                    kt = ib * Q + q
                    a = apool.tile([128, BC], F32, tag="a")
                    nc.scalar.activation(a[:], xs[ib][:], ACTF.Relu,
                                         bias=qb_s[:, q:q + 1], scale=1.0)
                    p = ppool.tile([128, BC], F32R, tag="p")
                    nc.vector._custom_dve(D1CUBE, out=p[:], in0=a[:], s0=3.0)
                    if ib < IB - 1:
                        rhs = h2_s[:, kt, :]
                        for bb in range(NBB):
                            nc.tensor.matmul(psums[bb][:],
                                             p[:, bb * 128:(bb + 1) * 128],
                                             rhs,
                                             start=False, stop=False)
                    else:
                        planes[q] = p

            # last i-block: bank-major so each PSUM bank finishes ~1.6us
            # apart and its drain + out-DMA overlaps the remaining matmuls.
            for bb in range(NBB):
                for q in range(Q):
                    kt = (IB - 1) * Q + q
                    nc.tensor.matmul(psums[bb][:],
                                     planes[q][:, bb * 128:(bb + 1) * 128],
                                     h2_s[:, kt, :],
                                     start=False, stop=(q == Q - 1))
                if bb in (1, 3, 5):
                    j = bb // 2
                    o2 = opool.tile([128, 2, O], F32, tag="o", bufs=2)
                    nc.scalar.copy(o2[:, 0, :], psums[2 * j][:])
                    nc.vector.tensor_copy(out=o2[:, 1, :],
                                          in_=psums[2 * j + 1][:])
                    nc.sync.dma_start(
                        out=out_d[2 * j * 128:(2 * j + 2) * 128, :].rearrange(
                            "(k p) o -> p k o", p=128),
                        in_=o2[:])
                elif bb == 6:
                    o6 = opool.tile([128, O], F32, tag="o1", bufs=1)
                    nc.scalar.copy(o6[:], psums[6][:])
                    nc.scalar.dma_start(out=out_d[6 * 128:7 * 128, :],
                                        in_=o6[:])
                elif bb == 7:
                    o7 = opool.tile([128, O], F32, tag="o2", bufs=1)
                    nc.vector.tensor_copy(out=o7[:], in_=psums[7][:])
                    nc.sync.dma_start(out=out_d[7 * 128:8 * 128, :],
                                      in_=o7[:])

    nc.compile()
    return nc


def _get_program():
    if "p" not in _programs:
        _programs["p"] = _build_program()
    return _programs["p"]


def _host_prep(x, weights, coefficients):
    x = np.asarray(x, dtype=np.float32)
    weights = np.asarray(weights, dtype=np.float32)
    coefficients = np.asarray(coefficients, dtype=np.float32)

    # raw truncated-power coefficients G_q = sum_g w5[q-g]/6 * C2_g
    c2 = coefficients.astype(np.float64) * weights.astype(np.float64)[:, :, None]
    c2 = c2.transpose(2, 1, 0)                     # [G, I, O]
    w5 = np.array([1.0, -4.0, 6.0, -4.0, 1.0]) / 6.0
    graw = np.zeros((Q, I, O), dtype=np.float64)
    for q in range(Q):
        for g in range(G):
            r = q - g
            if 0 <= r <= 4:
                graw[q] += w5[r] * c2[g]
    # planes P_q = D1_q = V_q - V_{q+1} (V_7 := 0)  =>  H = A^{-T} G
    A = np.eye(Q)
    A[np.arange(Q - 1), np.arange(1, Q)] = -1.0
    h = np.einsum('pq,qio->pio', np.linalg.inv(A).T, graw)
    # device row order kt = ib*7 + q
    h2k = np.empty((KT, 128, O), dtype=np.float32)
    for ib in range(IB):
        for q in range(Q):
            h2k[ib * Q + q] = h[q, ib * 128:(ib + 1) * 128, :]
    h2k = np.ascontiguousarray(h2k.reshape(KT * 128, O))

    # tp = min(3.5*x, 3.5): t<0 needs no clamp (relu zeroes every plane)
    tp = np.minimum(3.5 * x, np.float32(3.5)).astype(np.float32)
    xt = np.ascontiguousarray(tp.T)                # [I, B]
    qb = np.tile((3.5 - np.arange(8, dtype=np.float32))[None, :], (128, 1))

    in_maps = []
    for c in range(N_CORES):
        in_maps.append({
            "xt": np.ascontiguousarray(xt[:, c * BC:(c + 1) * BC]),
            "h2": h2k,
            "qb": qb,
        })
    return in_maps


def _run(x, weights, coefficients, **spmd_kwargs):
    nc = _get_program()
    in_maps = _host_prep(x, weights, coefficients)
    res = run_bass_kernel_spmd(nc, in_maps, list(range(N_CORES)), **spmd_kwargs)
    out = np.concatenate([res.results[c]["out"] for c in range(N_CORES)], axis=0)
    return out.astype(np.float32), res


def kernel(x, weights, coefficients):
    out, _ = _run(x, weights, coefficients)
    return out


# revision 15
# speedup vs baseline: 1.0532x; 1.0532x over previous
"""Trainium2 Bass kernel for nn_CustomBSplineLayer.

Math: out[b,o] = sum_{i,g} coeff[o,i,g] * w[o,i] * s_g(clip(x[b,i], -1, 1))
where s_g is a cubic B-spline basis on uniform knots (t = 3.5*(x+1) in [0,7],
8 basis functions; s_7 == 0 on the clipped domain).

Uniform-knot truncated-power identity: with V_q = relu(t-q)^3, the layer is
out = sum_{q,i} P_q[b,i] * H[(q,i), o] for ANY plane basis P spanning {V_q}
(H solved exactly on host).  The PE runs float32r (full rate) which rounds
each product at ~2^-12 relative, so per-plane error scales with |P_q|*|H_q|.
First-difference planes D1_q = V_q - V_{q+1} (<=127; V_7 := 0) measure ~5e-3
relative output error -- well under the 2e-2 gate -- and have a key property:

    D1_q is a function of a_q = relu(t-q) ALONE:
        D1_q = m^3 + 3*(a_q - m)*a_q,   m = min(a_q, 1)
    (for t>=q+1 this is 3s^2-3s+1 with s=t-q; for t in [q,q+1] it's s^3).

So each plane needs exactly TWO on-chip ops: one ScalarE relu (free bias
shift) and one fused custom DVE instruction (D1CUBE_ANT, registered below,
5 ALU stages).  No folds, no gpsimd, no squares: per i-block the old kernel's
~30 elementwise ops become 14.  The clamp tp = min(3.5x, 3.5) is precomputed
on host (t<0 needs no clamp: every plane vanishes there via the relu).

Layout (data-parallel over batch, 8 cores x 1024 rows):
  - xt = host tp, pre-transposed: [512 i, 1024 b] per core, i on partitions.
  - planes per (i-block, q): [128, 1024] f32r tiles; matmul lhsT slices are
    [128 K, 128 M=batch] column windows; rhs H tiles [128, 512 o] (f32r).
  - h2 DMA'd in 28 per-kt chunks so the first matmul can start ~1us in.
  - PSUM [128 b, 512 o] x 8 banks accumulate all 28 k-tiles.
"""

import numpy as np

import concourse.mybir as mybir
from concourse import bacc
import concourse.tile as tile
from concourse.bass_utils import run_bass_kernel_spmd
from concourse import dve_ops as _dops
from concourse.dve_spec import Spec, Src0, C0, One, minn, sq
from concourse.dve_spec import lower as _dve_lower
from concourse.dve_uop import DveOpSpec as _DveOpSpec

F32 = mybir.dt.float32
F32R = mybir.dt.float32r
ACTF = mybir.ActivationFunctionType

N_CORES = 8
BATCH, I, O, G = 8192, 512, 512, 8
BC = BATCH // N_CORES          # 1024 batch rows per core
Q = 7                          # planes q = 0..6
IB = I // 128                  # 4 i-blocks
KT = Q * IB                    # 28 k-tiles of 128
NBB = BC // 128                # 8 batch blocks of 128


def _register_d1cube():
    """Register the fused plane op: out = m^3 + s0*(a-m)*a, m = min(a, 1).

    With a = relu(t-q) >= 0 and s0 = 3.0 this is exactly
    D1_q(t) = relu(t-q)^3 - relu(t-q-1)^3 for t <= q+... (all t; V_{q+1}
    is a function of a_q since relu(t-q-1) = relu(a_q - 1))."""
    name = "D1CUBE_ANT"
    for op in _dops.OPS:
        if op.name == name:
            return op

    def _ref(in0, in1, s0, s1, imm2):
        a = in0.astype(np.float32)
        m = np.minimum(a, np.float32(1.0))
        return (m * m * m + (a - m) * a * np.float32(s0)).astype(np.float32)

    m = minn(Src0, One)
    spec = Spec(body=sq(m) * m + (Src0 - m) * Src0 * C0, reference=_ref)
    opcode = _dops._CUSTOM_DVE_ROW_BASE + len(_dops.OPS)
    assert opcode < 0x20
    shas = {}
    for ver in ("v3", "v4"):
        try:
            shas[ver] = _DveOpSpec(
                name=name, opcode=opcode, uops=_dve_lower(spec, ver=ver),
                rd1_en=False).sha(ver)
        except Exception:
            pass
    op = _dops.DveOp(name, spec, subdim=False, uops_sha=shas)
    _dops.OPS.append(op)
    _dops.CUSTOM_DVE_SPECS[name] = spec
    _dops._SUB_OPCODE_FOR_NAME[name] = opcode
    return op


D1CUBE = _register_d1cube()

_programs = {}


def _build_program():
    nc = bacc.Bacc("TRN2", target_bir_lowering=False, debug=False,
                   num_devices=N_CORES)
    xt_d = nc.dram_tensor("xt", [I, BC], F32, kind="ExternalInput").ap()
    h2_d = nc.dram_tensor("h2", [KT * 128, O], F32R, kind="ExternalInput").ap()
    qb_d = nc.dram_tensor("qb", [128, 8], F32, kind="ExternalInput").ap()
    out_d = nc.dram_tensor("out", [BC, O], F32, kind="ExternalOutput").ap()

    with tile.TileContext(nc) as tc:
        with tc.tile_pool(name="g", bufs=1) as gpool, \
             tc.tile_pool(name="x", bufs=4) as xpool, \
             tc.tile_pool(name="a", bufs=4) as apool, \
             tc.tile_pool(name="p", bufs=9) as ppool, \
             tc.tile_pool(name="o", bufs=4) as opool, \
             tc.tile_pool(name="ps", bufs=1, space="PSUM") as pspool:

            # warm tile: memset'd (no DMA dep) -- feeds the scalar table-load
            # hoist and the PE HAM warm-up matmuls before real data lands.
            # f32r so the dummies are single-pass (fp32 lowers to 2x LOW_HIGH).
            warm0 = gpool.tile([128, 512], F32)
            nc.gpsimd.memset(warm0[:], 0.0)
            warm = gpool.tile([128, 512], F32R)
            nc.vector.tensor_copy(out=warm[:], in_=warm0[:])
            scr = gpool.tile([128, 8], F32)

            qb_s = gpool.tile([128, 8], F32)
            xs = [xpool.tile([128, BC], F32, name=f"xs{ib}", tag=f"xs{ib}")
                  for ib in range(IB)]

            # scalar queue (q10) carries ONLY xs0, as two column-halves so
            # the A-half pipeline starts ~1.5us before the full tile lands;
            # the dummy activation pulls ACT_TABLE_LOAD ahead while xs0 flies.
            nc.scalar.dma_start(out=xs[0][:, 0:BC // 2],
                                in_=xt_d[0:128, 0:BC // 2])
            nc.scalar.dma_start(out=xs[0][:, BC // 2:BC],
                                in_=xt_d[0:128, BC // 2:BC])
            nc.scalar.activation(scr[:], warm0[:, 0:8], ACTF.Relu, scale=1.0)

            psums = [pspool.tile([128, O], F32, name=f"ps{bb}", tag=f"ps{bb}")
                     for bb in range(NBB)]

            # PE HAM warm-up: ~10 single-pass f32r dummies (~4.3us cold) keep
            # the PE busy from ~7.2us until the first real matmul, so the
            # clock gate opens at ~10.6us and the real stream runs at 2.4GHz.
            for _ in range(8):
                nc.tensor.matmul(psums[0][:], warm[:, 0:128], warm[:],
                                 start=True, stop=True)

            # sync queue: h2 finely chunked up front, then coarse; xs1-3
            # interleaved where their deadlines fall.
            h2_s = gpool.tile([128, KT, O], F32R)

            def h2_dma(k0, k1):
                nc.sync.dma_start(
                    out=h2_s[:, k0:k1, :],
                    in_=h2_d[k0 * 128:k1 * 128, :].rearrange(
                        "(kt p) o -> p kt o", p=128))

            nc.sync.dma_start(out=qb_s[:], in_=qb_d[:])
            h2_dma(0, 1)
            h2_dma(1, 2)
            nc.sync.dma_start(out=xs[1][:], in_=xt_d[128:256, :])
            h2_dma(2, 7)
            nc.sync.dma_start(out=xs[2][:], in_=xt_d[256:384, :])
            nc.sync.dma_start(out=xs[3][:], in_=xt_d[384:512, :])
            h2_dma(7, 14)
            h2_dma(14, 21)
            h2_dma(21, 28)

            # i-blocks 0..2: plane-major (kt inner order), PSUM-bank inner.
            planes = {}
            # Planes 0-1 in column-halves, interleaved [A0, A1, B0, B1] with
            # matmul groups [kt0:bb0-3][kt1:bb0-3][kt0:bb4-7][kt1:bb4-7]:
            # the A halves only need the first half of xs0, so the PE starts
            # ~1.5us earlier; from plane 2 on, full-width planes keep the PE
            # consumption rate (1.8us/plane) above the production rate.
            # Bank accumulation only needs kt0 started first PER BANK.
            hp = {}
            for hh, q in ((0, 0), (0, 1), (1, 0), (1, 1)):
                cols = slice(hh * BC // 2, (hh + 1) * BC // 2)
                ah = apool.tile([128, BC // 2], F32, tag="a0", bufs=2)
                nc.scalar.activation(ah[:], xs[0][:, cols], ACTF.Relu,
                                     bias=qb_s[:, q:q + 1], scale=1.0)
                ph = ppool.tile([128, BC // 2], F32R, tag="p0", bufs=2)
                nc.vector._custom_dve(D1CUBE, out=ph[:], in0=ah[:], s0=3.0)
                for b4 in range(4):
                    bb = hh * 4 + b4
                    nc.tensor.matmul(psums[bb][:],
                                     ph[:, b4 * 128:(b4 + 1) * 128],
                                     h2_s[:, q, :],
                                     start=(q == 0), stop=False)

            for ib in range(IB):
                for q in range(Q):
                    if ib == 0 and q < 2:
                        continue
                    kt = ib * Q + q
                    a = apool.tile([128, BC], F32, tag="a")
                    nc.scalar.activation(a[:], xs[ib][:], ACTF.Relu,
                                         bias=qb_s[:, q:q + 1], scale=1.0)
                    p = ppool.tile([128, BC], F32R, tag="p")
                    nc.vector._custom_dve(D1CUBE, out=p[:], in0=a[:], s0=3.0)
                    if ib < IB - 1:
                        rhs = h2_s[:, kt, :]
                        for bb in range(NBB):
                            nc.tensor.matmul(psums[bb][:],
                                             p[:, bb * 128:(bb + 1) * 128],
                                             rhs,
                                             start=False, stop=False)
                    else:
                        planes[q] = p

            # last i-block: bank-major so each PSUM bank finishes ~1.6us
            # apart and its drain + out-DMA overlaps the remaining matmuls.
            for bb in range(NBB):
                for q in range(Q):
                    kt = (IB - 1) * Q + q
                    nc.tensor.matmul(psums[bb][:],
                                     planes[q][:, bb * 128:(bb + 1) * 128],
                                     h2_s[:, kt, :],
                                     start=False, stop=(q == Q - 1))
                if bb in (1, 3, 5):
                    j = bb // 2
                    o2 = opool.tile([128, 2, O], F32, tag="o", bufs=2)
                    nc.scalar.copy(o2[:, 0, :], psums[2 * j][:])
                    nc.vector.tensor_copy(out=o2[:, 1, :],
                                          in_=psums[2 * j + 1][:])
                    nc.sync.dma_start(
                        out=out_d[2 * j * 128:(2 * j + 2) * 128, :].rearrange(
                            "(k p) o -> p k o", p=128),
                        in_=o2[:])
                elif bb == 6:
                    o6 = opool.tile([128, O], F32, tag="o1", bufs=1)
                    nc.scalar.copy(o6[:], psums[6][:])
                    nc.scalar.dma_start(out=out_d[6 * 128:7 * 128, :],
                                        in_=o6[:])
                elif bb == 7:
                    o7 = opool.tile([128, O], F32, tag="o2", bufs=1)
                    nc.vector.tensor_copy(out=o7[:], in_=psums[7][:])
                    nc.sync.dma_start(out=out_d[7 * 128:8 * 128, :],
                                      in_=o7[:])

    nc.compile()
    return nc


def _get_program():
    if "p" not in _programs:
        _programs["p"] = _build_program()
    return _programs["p"]


def _host_prep(x, weights, coefficients):
    x = np.asarray(x, dtype=np.float32)
    weights = np.asarray(weights, dtype=np.float32)
    coefficients = np.asarray(coefficients, dtype=np.float32)

    # raw truncated-power coefficients G_q = sum_g w5[q-g]/6 * C2_g
    c2 = coefficients.astype(np.float64) * weights.astype(np.float64)[:, :, None]
    c2 = c2.transpose(2, 1, 0)                     # [G, I, O]
    w5 = np.array([1.0, -4.0, 6.0, -4.0, 1.0]) / 6.0
    graw = np.zeros((Q, I, O), dtype=np.float64)
    for q in range(Q):
        for g in range(G):
            r = q - g
            if 0 <= r <= 4:
                graw[q] += w5[r] * c2[g]
    # planes P_q = D1_q = V_q - V_{q+1} (V_7 := 0)  =>  H = A^{-T} G
    A = np.eye(Q)
    A[np.arange(Q - 1), np.arange(1, Q)] = -1.0
    h = np.einsum('pq,qio->pio', np.linalg.inv(A).T, graw)
    # device row order kt = ib*7 + q
    h2k = np.empty((KT, 128, O), dtype=np.float32)
    for ib in range(IB):
        for q in range(Q):
            h2k[ib * Q + q] = h[q, ib * 128:(ib + 1) * 128, :]
    h2k = np.ascontiguousarray(h2k.reshape(KT * 128, O))

    # tp = min(3.5*x, 3.5): t<0 needs no clamp (relu zeroes every plane)
    tp = np.minimum(3.5 * x, np.float32(3.5)).astype(np.float32)
    xt = np.ascontiguousarray(tp.T)                # [I, B]
    qb = np.tile((3.5 - np.arange(8, dtype=np.float32))[None, :], (128, 1))

    in_maps = []
    for c in range(N_CORES):
        in_maps.append({
            "xt": np.ascontiguousarray(xt[:, c * BC:(c + 1) * BC]),
            "h2": h2k,
            "qb": qb,
        })
    return in_maps


def _run(x, weights, coefficients, **spmd_kwargs):
    nc = _get_program()
    in_maps = _host_prep(x, weights, coefficients)
    res = run_bass_kernel_spmd(nc, in_maps, list(range(N_CORES)), **spmd_kwargs)
    out = np.concatenate([res.results[c]["out"] for c in range(N_CORES)], axis=0)
    return out.astype(np.float32), res


def kernel(x, weights, coefficients):
    out, _ = _run(x, weights, coefficients)
    return out
